# revision 11
# baseline (speedup 1.0000x reference)
"""Trainium2 Bass kernel for nn_AttentionMLP: per-sample 16-head attention over
N=1024 tokens with mean-pooling + LayerNorm.  Data-parallel over batch across
8 NeuronCores (4 samples/core).

Key algebraic restructuring: the reference computes
    out = mean_i( softmax(q_i K^T * s) @ V );  y = LN(out)
By linearity of the mean, with e[i,j] = exp(s * S[i,j]) and den[i] = sum_j e[i,j]:
    out = (1/N) * (sum_i e[i,:] / den[i]) @ V = (1/N) * w @ V
so the [N,N]@[N,64] attention-value matmul collapses to a rank-1 reduction
(w = r^T @ e, an M=1 matmul on the PE) plus one [1,N]@[N,64] product.
The exp of all N^2 scores (the unavoidable cost) runs on the scalar engine
with the fused per-row accumulate (accum_out) producing den for free.

Precision: matmuls run in bf16 (fp32 runs the PE at ~5x lower effective
throughput: 2 HW passes x half stream rate); PSUM accumulation, den,
reciprocal and the LayerNorm are fp32.  Errors injected on exp/r average
out over the 1024-token reduction before reaching the output.

Layouts (per core):
  x_sb  [128c, 5ct, 1024i]   (c = ct*128 + p), straight from DRAM
  qT/kT per head-pair [128e', 1024i] via matmul(lhsT=W*T[c,e], rhs=x[c,i])
  scores S[i,j] psum [128, 1024] per (head, i-tile); 2 heads packed in
  distinct PE row groups (K=64).  w accumulated in psum [1,1024] rows at
  col-group 0/32 (concurrent).  V[j,e] per sample, fin = wT^T @ V.
"""

import numpy as np

HEADS = 16
HEAD_DIM = 64
B, C, HW = 32, 640, 1024
N_CORES = 8
B_LOC = B // N_CORES      # 4 samples per core
CT = C // 128             # 5 contraction tiles
NT = HW // 128            # 8 token tiles
HP = HEADS // 2           # 8 head pairs
INNER = HEADS * HEAD_DIM  # 1024
LN_EPS = 1e-5
SCALE = HEAD_DIM ** -0.5

_CACHE = {}


def _build_module():
    from contextlib import ExitStack
    import concourse.bacc as bacc
    import concourse.mybir as mybir
    import concourse.tile as tile
    from concourse import masks

    f32 = mybir.dt.float32
    bf16 = mybir.dt.bfloat16
    AF = mybir.ActivationFunctionType
    Alu = mybir.AluOpType

    nc = bacc.Bacc("TRN2", debug=False, enable_asserts=False)

    x_d = nc.dram_tensor("x", [B_LOC, C, HW], bf16, kind="ExternalInput").ap()
    wq_d = nc.dram_tensor("wqT", [C, INNER], bf16, kind="ExternalInput").ap()
    wk_d = nc.dram_tensor("wkT", [C, INNER], bf16, kind="ExternalInput").ap()
    wv_d = nc.dram_tensor("wvT", [C, INNER], bf16, kind="ExternalInput").ap()
    gam_d = nc.dram_tensor("gamma2d", [B_LOC * HEADS, HEAD_DIM], f32,
                           kind="ExternalInput").ap()
    bet_d = nc.dram_tensor("beta2d", [B_LOC * HEADS, HEAD_DIM], f32,
                           kind="ExternalInput").ap()
    y_d = nc.dram_tensor("y", [B_LOC * HEADS, HEAD_DIM], f32,
                         kind="ExternalOutput").ap()

    with tile.TileContext(nc) as tc, ExitStack() as ctx:
        wts = ctx.enter_context(tc.tile_pool(name="wts", bufs=1))
        xp = ctx.enter_context(tc.tile_pool(name="xp", bufs=2))
        vp = ctx.enter_context(tc.tile_pool(name="vp", bufs=1))
        qkp = ctx.enter_context(tc.tile_pool(name="qkp", bufs=2))
        ep = ctx.enter_context(tc.tile_pool(name="ep", bufs=6))
        sp = ctx.enter_context(tc.tile_pool(name="sp", bufs=4))
        # scores double-buffer: 2 x [128,1024]f32 = 4 psum banks
        psb = ctx.enter_context(tc.tile_pool(name="psb", bufs=2, space="PSUM"))
        # everything else (projections / w / transposes / final): 2 x 2 banks
        pss = ctx.enter_context(tc.tile_pool(name="pss", bufs=2, space="PSUM"))

        # ---- constants / weights ----
        wq_sb = wts.tile([128, CT, INNER], bf16, tag="wq", name="wq_sb")
        wk_sb = wts.tile([128, CT, INNER], bf16, tag="wk", name="wk_sb")
        wv_sb = wts.tile([128, CT, INNER], bf16, tag="wv", name="wv_sb")
        for wsb, wd in ((wq_sb, wq_d), (wk_sb, wk_d), (wv_sb, wv_d)):
            wr = wd.rearrange("(ct p) e -> ct p e", p=128)
            for ct in range(CT):
                nc.sync.dma_start(out=wsb[:, ct], in_=wr[ct])

        ident = wts.tile([16, 16], bf16, tag="ident", name="ident")
        masks.make_identity(nc, ident[:])
        # (engine APs must start at a partition multiple of 32; per-head row
        # scatter/gather below therefore goes through SBUF->SBUF DMA)
        gam_sb = wts.tile([B_LOC * HEADS, HEAD_DIM], f32, tag="gam", name="gam_sb")
        bet_sb = wts.tile([B_LOC * HEADS, HEAD_DIM], f32, tag="bet", name="bet_sb")
        nc.sync.dma_start(out=gam_sb[:], in_=gam_d)
        nc.sync.dma_start(out=bet_sb[:], in_=bet_d)
        eps_sb = wts.tile([B_LOC * HEADS, 1], f32, tag="eps", name="eps_sb")
        nc.vector.memset(eps_sb[:], LN_EPS)

        y_sb = wts.tile([B_LOC * HEADS, HEAD_DIM], f32, tag="y", name="y_sb")

        x_tiles = {}
        qt_tiles = {}
        kt_tiles = {}
        v_tiles = {}

        def emit_x(b):
            xs = xp.tile([128, CT, HW], bf16, tag="x", name=f"x{b}")
            xr = x_d[b].rearrange("(ct p) i -> ct p i", p=128)
            for ct in range(CT):
                nc.sync.dma_start(out=xs[:, ct], in_=xr[ct])
            x_tiles[b] = xs

        def emit_qk_proj(b, hp, wsb, which):
            """qT/kT for head pair hp of sample b: [128e', 1024i] in SBUF."""
            dst = qkp.tile([128, HW], bf16, tag=which, name=f"{which}{b}_{hp}")
            ps = pss.tile([128, HW], f32, tag="sm", name=f"ps_{which}{b}_{hp}")
            xs = x_tiles[b]
            for ih in range(2):
                for ct in range(CT):
                    nc.tensor.matmul(
                        ps[:, ih * 512:(ih + 1) * 512],
                        wsb[:, ct, hp * 128:(hp + 1) * 128],
                        xs[:, ct, ih * 512:(ih + 1) * 512],
                        start=(ct == 0), stop=(ct == CT - 1),
                    )
            nc.vector.tensor_copy(dst[:], ps[:])
            return dst

        def emit_v_proj(b, jt):
            """V[j,e] j-tile jt of sample b into v_tiles[b][:, jt]."""
            ps = pss.tile([128, INNER], f32, tag="sm", name=f"ps_v{b}_{jt}")
            xs = x_tiles[b]
            for eh in range(2):
                for ct in range(CT):
                    nc.tensor.matmul(
                        ps[:, eh * 512:(eh + 1) * 512],
                        xs[:, ct, jt * 128:(jt + 1) * 128],
                        wv_sb[:, ct, eh * 512:(eh + 1) * 512],
                        start=(ct == 0), stop=(ct == CT - 1),
                    )
            nc.vector.tensor_copy(v_tiles[b][:, jt], ps[:])

        def emit_tail_transposes(b, w_rows):
            wT = sp.tile([128, NT, HEADS], bf16, tag="wt", bufs=2, name=f"wT{b}")
            for jt in range(NT):
                tp = pss.tile([128, HEADS], bf16, tag="sm", name=f"tp{b}_{jt}")
                nc.tensor.transpose(tp[:], w_rows[:, jt * 128:(jt + 1) * 128],
                                    ident[:])
                nc.vector.tensor_copy(wT[:, jt], tp[:])
            return wT

        def emit_tail_fin(b, wT):
            fin = pss.tile([HEADS, INNER], f32, tag="sm", name=f"fin{b}")
            for eh in range(2):
                for jt in range(NT):
                    nc.tensor.matmul(
                        fin[:, eh * 512:(eh + 1) * 512],
                        wT[:, jt],
                        v_tiles[b][:, jt, eh * 512:(eh + 1) * 512],
                        start=(jt == 0), stop=(jt == NT - 1),
                    )
            fin_sb = sp.tile([HEADS, INNER], f32, tag="finsb", bufs=2,
                             name=f"finsb{b}")
            nc.vector.tensor_scalar_mul(fin_sb[:], fin[:], 1.0 / HW)
            for h in range(HEADS):
                nc.sync.dma_start(
                    out=y_sb[b * HEADS + h:b * HEADS + h + 1, :],
                    in_=fin_sb[h:h + 1, h * HEAD_DIM:(h + 1) * HEAD_DIM])
            del v_tiles[b]

        # ---- startup ----
        emit_x(0)
        qt_tiles[(0, 0)] = emit_qk_proj(0, 0, wq_sb, "qt")
        kt_tiles[(0, 0)] = emit_qk_proj(0, 0, wk_sb, "kt")

        w_rows_of = {}
        for b in range(B_LOC):
            v_tiles[b] = vp.tile([128, NT, INNER], bf16, tag="v", bufs=2,
                                 name=f"v{b}")
            w_rows = sp.tile([HEADS, HW], bf16, tag="wr", bufs=2, name=f"wr{b}")
            w_rows_of[b] = w_rows
            for hp in range(HP):
                qt = qt_tiles.pop((b, hp))
                kt = kt_tiles.pop((b, hp))
                w_ps = pss.tile([128, HW], f32, tag="sm", name=f"w{b}_{hp}")
                # next pair to prefetch (same sample, or first pair of next)
                if hp + 1 < HP:
                    nxt = (b, hp + 1)
                elif b + 1 < B_LOC:
                    nxt = (b + 1, 0)
                else:
                    nxt = None
                for it in range(NT):
                    # --- prefetch / tail injections, never at it==0 so the
                    # pair's first scores reach ACT immediately ---
                    if it == 1 and nxt is not None:
                        qt_tiles[nxt] = emit_qk_proj(nxt[0], nxt[1], wq_sb, "qt")
                    if it == 2 and hp == 0 and b + 1 < B_LOC:
                        emit_x(b + 1)
                    if it == 3 and nxt is not None:
                        kt_tiles[nxt] = emit_qk_proj(nxt[0], nxt[1], wk_sb, "kt")
                    if it == 4 and hp >= 1:
                        emit_v_proj(b, jt=hp - 1)
                    if it == 6 and hp == HP - 1:
                        emit_v_proj(b, jt=NT - 1)
                    # previous sample's tail hides inside this sample's pair 0
                    if hp == 0 and b >= 1:
                        if it == 5:
                            wT_prev = emit_tail_transposes(b - 1, w_rows_of[b - 1])
                        if it == 7:
                            emit_tail_fin(b - 1, wT_prev)
                    # --- scores for both heads (distinct PE row groups) ---
                    s0 = psb.tile([128, HW], f32, tag="big", name=f"s0_{b}_{hp}_{it}")
                    s1 = psb.tile([128, HW], f32, tag="big", name=f"s1_{b}_{hp}_{it}")
                    # alternate heads so each MM overlaps its row-group partner
                    for jh in range(2):
                        for h, s in ((0, s0), (1, s1)):
                            nc.tensor.matmul(
                                s[:, jh * 512:(jh + 1) * 512],
                                qt[h * 64:(h + 1) * 64, it * 128:(it + 1) * 128],
                                kt[h * 64:(h + 1) * 64, jh * 512:(jh + 1) * 512],
                                start=True, stop=True,
                            )
                    # --- exp (+row-sum) then w += r^T @ e ---
                    # (h, jh) half goes to psum row 32*(2h+jh): 4 distinct PE
                    # column groups, so all four M=1 matmuls run concurrently
                    for h, s in ((0, s0), (1, s1)):
                        ex = ep.tile([128, HW], bf16, tag="e", name=f"e{b}_{hp}_{h}_{it}")
                        den = sp.tile([128, 1], f32, tag="den", name=f"den{b}_{hp}_{h}_{it}")
                        r = sp.tile([128, 1], f32, tag="r", name=f"r{b}_{hp}_{h}_{it}")
                        rb = sp.tile([128, 1], bf16, tag="rb", name=f"rb{b}_{hp}_{h}_{it}")
                        nc.scalar.activation(ex[:], s[:], AF.Exp,
                                             scale=SCALE, accum_out=den[:])
                        nc.vector.reciprocal(r[:], den[:])
                        nc.vector.tensor_copy(rb[:], r[:])
                        for jh in range(2):
                            row = 32 * (2 * h + jh)
                            nc.tensor.matmul(
                                w_ps[row:row + 1, jh * 512:(jh + 1) * 512],
                                rb[:],
                                ex[:, jh * 512:(jh + 1) * 512],
                                start=(it == 0), stop=(it == NT - 1),
                                skip_group_check=True,
                                tile_position=(0, row),
                            )
                stage = sp.tile([128, HW], bf16, tag="wstage", bufs=2,
                                name=f"wstage{b}_{hp}")
                nc.vector.tensor_copy(stage[:], w_ps[:, :])
                for h in range(2):
                    for jh in range(2):
                        row = 32 * (2 * h + jh)
                        nc.sync.dma_start(
                            out=w_rows[2 * hp + h:2 * hp + h + 1,
                                       jh * 512:(jh + 1) * 512],
                            in_=stage[row:row + 1, jh * 512:(jh + 1) * 512])

        # last sample's tail (nothing left to hide it behind)
        wT_last = emit_tail_transposes(B_LOC - 1, w_rows_of[B_LOC - 1])
        emit_tail_fin(B_LOC - 1, wT_last)

        # ---- LayerNorm over last dim (64) for all 64 (b,h) rows ----
        P = B_LOC * HEADS
        stats = sp.tile([P, 6], f32, tag="st", bufs=1, name="stats")
        mv = sp.tile([P, 2], f32, tag="mv", bufs=1, name="mv")
        std = sp.tile([P, 1], f32, tag="sd", bufs=1, name="std")
        nc.vector.bn_stats(stats[:], y_sb[:])
        nc.vector.bn_aggr(mv[:], stats[:])
        nc.scalar.activation(std[:], mv[:, 1:2], AF.Sqrt,
                             bias=eps_sb[:], scale=1.0)
        nc.vector.reciprocal(std[:], std[:])
        nc.vector.tensor_scalar(y_sb[:], y_sb[:], mv[:, 0:1], std[:],
                                op0=Alu.subtract, op1=Alu.mult)
        nc.vector.tensor_mul(y_sb[:], y_sb[:], gam_sb[:])
        nc.vector.tensor_add(y_sb[:], y_sb[:], bet_sb[:])
        nc.sync.dma_start(out=y_d, in_=y_sb[:])

    nc.compile()
    return nc


def _get_nc():
    if "nc" not in _CACHE:
        _CACHE["nc"] = _build_module()
    return _CACHE["nc"]


def _prep_in_maps(x, Wq, Wk, Wv, gamma, beta):
    import ml_dtypes
    bf = ml_dtypes.bfloat16
    x = np.asarray(x, np.float32)
    wqT = np.ascontiguousarray(np.asarray(Wq, np.float32).T.astype(bf))
    wkT = np.ascontiguousarray(np.asarray(Wk, np.float32).T.astype(bf))
    wvT = np.ascontiguousarray(np.asarray(Wv, np.float32).T.astype(bf))
    gam2 = np.ascontiguousarray(
        np.broadcast_to(np.asarray(gamma, np.float32), (B_LOC * HEADS, HEAD_DIM)))
    bet2 = np.ascontiguousarray(
        np.broadcast_to(np.asarray(beta, np.float32), (B_LOC * HEADS, HEAD_DIM)))
    in_maps = []
    for c in range(N_CORES):
        xb = np.ascontiguousarray(
            x[c * B_LOC:(c + 1) * B_LOC].reshape(B_LOC, C, HW).astype(bf))
        in_maps.append(dict(x=xb, wqT=wqT, wkT=wkT, wvT=wvT,
                            gamma2d=gam2, beta2d=bet2))
    return in_maps


def _run(inputs, trace=False):
    from concourse.bass_utils import run_bass_kernel_spmd
    nc = _get_nc()
    in_maps = _prep_in_maps(**inputs)
    res = run_bass_kernel_spmd(nc, in_maps, core_ids=list(range(N_CORES)),
                               trace=trace)
    out = np.concatenate(
        [np.asarray(res.results[c]["y"], np.float32).reshape(B_LOC, HEADS, HEAD_DIM)
         for c in range(N_CORES)],
        axis=0)
    return out, res


def kernel(x, Wq, Wk, Wv, gamma, beta):
    out, _ = _run(dict(x=x, Wq=Wq, Wk=Wk, Wv=Wv, gamma=gamma, beta=beta))
    return out


# revision 17
# speedup vs baseline: 1.3355x; 1.3355x over previous
"""Trainium2 Bass kernel for nn_AttentionMLP: per-sample 16-head attention over
N=1024 tokens with mean-pooling + LayerNorm.  Data-parallel over batch across
8 NeuronCores (4 samples/core).

Key algebraic restructuring: the reference computes
    out = mean_i( softmax(q_i K^T * s) @ V );  y = LN(out)
By linearity of the mean, with e[i,j] = exp(s * S[i,j]) and den[i] = sum_j e[i,j]:
    out = (1/N) * (sum_i e[i,:] / den[i]) @ V = (1/N) * w @ V
so the [N,N]@[N,64] attention-value matmul collapses to a rank-1 reduction
(w = r^T @ e, an M=1 matmul on the PE) plus one [1,N]@[N,64] product.
The exp of all N^2 scores (the unavoidable cost) runs on the scalar engine
with the fused per-row accumulate (accum_out) producing den for free.

Precision: matmuls run in bf16 (fp32 runs the PE at ~5x lower effective
throughput: 2 HW passes x half stream rate); PSUM accumulation, den,
reciprocal and the LayerNorm are fp32.  Errors injected on exp/r average
out over the 1024-token reduction before reaching the output.

Layouts (per core):
  x_sb  [128c, 5ct, 1024i]   (c = ct*128 + p), straight from DRAM
  qT/kT per head-pair [128e', 1024i] via matmul(lhsT=W*T[c,e], rhs=x[c,i])
  scores S[i,j] psum [128, 1024] per (head, i-tile); 2 heads packed in
  distinct PE row groups (K=64).  w accumulated in psum [1,1024] rows at
  col-group 0/32 (concurrent).  V[j,e] per sample, fin = wT^T @ V.
"""

import numpy as np

HEADS = 16
HEAD_DIM = 64
B, C, HW = 32, 640, 1024
N_CORES = 8
B_LOC = B // N_CORES      # 4 samples per core
CT = C // 128             # 5 contraction tiles
NT = HW // 128            # 8 token tiles
HP = HEADS // 2           # 8 head pairs
INNER = HEADS * HEAD_DIM  # 1024
LN_EPS = 1e-5
SCALE = HEAD_DIM ** -0.5

_CACHE = {}


def _build_module():
    from contextlib import ExitStack
    import concourse.bacc as bacc
    import concourse.mybir as mybir
    import concourse.tile as tile
    from concourse import masks

    f32 = mybir.dt.float32
    bf16 = mybir.dt.bfloat16
    AF = mybir.ActivationFunctionType
    Alu = mybir.AluOpType

    nc = bacc.Bacc("TRN2", debug=False, enable_asserts=False)

    x_d = nc.dram_tensor("x", [B_LOC, C, HW], bf16, kind="ExternalInput").ap()
    wq_d = nc.dram_tensor("wqT", [C, INNER], bf16, kind="ExternalInput").ap()
    wk_d = nc.dram_tensor("wkT", [C, INNER], bf16, kind="ExternalInput").ap()
    wv_d = nc.dram_tensor("wvT", [C, INNER], bf16, kind="ExternalInput").ap()
    gam_d = nc.dram_tensor("gamma2d", [B_LOC * HEADS, HEAD_DIM], f32,
                           kind="ExternalInput").ap()
    bet_d = nc.dram_tensor("beta2d", [B_LOC * HEADS, HEAD_DIM], f32,
                           kind="ExternalInput").ap()
    y_d = nc.dram_tensor("y", [B_LOC * HEADS, HEAD_DIM], f32,
                         kind="ExternalOutput").ap()

    with tile.TileContext(nc) as tc, ExitStack() as ctx:
        wts = ctx.enter_context(tc.tile_pool(name="wts", bufs=1))
        xp = ctx.enter_context(tc.tile_pool(name="xp", bufs=2))
        vp = ctx.enter_context(tc.tile_pool(name="vp", bufs=1))
        qkp = ctx.enter_context(tc.tile_pool(name="qkp", bufs=2))
        ep = ctx.enter_context(tc.tile_pool(name="ep", bufs=20))
        sp = ctx.enter_context(tc.tile_pool(name="sp", bufs=4))
        # scores triple-buffer (+ transient w block): 3 x [128,1024]f32 = 6 banks
        psb = ctx.enter_context(tc.tile_pool(name="psb", bufs=3, space="PSUM"))
        # projections / transposes / final: 1 x 2 banks
        pss = ctx.enter_context(tc.tile_pool(name="pss", bufs=1, space="PSUM"))

        # ---- constants / weights ----
        wq_sb = wts.tile([128, CT, INNER], bf16, tag="wq", name="wq_sb")
        wk_sb = wts.tile([128, CT, INNER], bf16, tag="wk", name="wk_sb")
        wv_sb = wts.tile([128, CT, INNER], bf16, tag="wv", name="wv_sb")
        for wsb, wd in ((wq_sb, wq_d), (wk_sb, wk_d), (wv_sb, wv_d)):
            wr = wd.rearrange("(ct p) e -> ct p e", p=128)
            for ct in range(CT):
                nc.sync.dma_start(out=wsb[:, ct], in_=wr[ct])

        ident = wts.tile([16, 16], bf16, tag="ident", name="ident")
        masks.make_identity(nc, ident[:])
        # (engine APs must start at a partition multiple of 32; per-head row
        # scatter/gather below therefore goes through SBUF->SBUF DMA)
        gam_sb = wts.tile([B_LOC * HEADS, HEAD_DIM], f32, tag="gam", name="gam_sb")
        bet_sb = wts.tile([B_LOC * HEADS, HEAD_DIM], f32, tag="bet", name="bet_sb")
        nc.sync.dma_start(out=gam_sb[:], in_=gam_d)
        nc.sync.dma_start(out=bet_sb[:], in_=bet_d)
        eps_sb = wts.tile([B_LOC * HEADS, 1], f32, tag="eps", name="eps_sb")
        nc.vector.memset(eps_sb[:], LN_EPS)

        y_sb = wts.tile([B_LOC * HEADS, HEAD_DIM], f32, tag="y", name="y_sb")

        x_tiles = {}
        qt_tiles = {}
        kt_tiles = {}
        v_tiles = {}

        def emit_x(b):
            xs = xp.tile([128, CT, HW], bf16, tag="x", name=f"x{b}")
            xr = x_d[b].rearrange("(ct p) i -> ct p i", p=128)
            for ct in range(CT):
                nc.sync.dma_start(out=xs[:, ct], in_=xr[ct])
            x_tiles[b] = xs

        proj_state = {}

        def emit_qk_proj_half(b, hp, wsb, which, ih):
            """Half (512 i-columns) of the qT/kT projection for pair hp.
            Emitted in two chunks so the PE detour never starves ACT."""
            key = (which, b, hp)
            if ih == 0:
                dst = qkp.tile([128, HW], bf16, tag=which, name=f"{which}{b}_{hp}")
                ps = pss.tile([128, HW], f32, tag="sm", name=f"ps_{which}{b}_{hp}")
                proj_state[key] = (dst, ps)
            dst, ps = proj_state[key]
            xs = x_tiles[b]
            for ct in range(CT):
                nc.tensor.matmul(
                    ps[:, ih * 512:(ih + 1) * 512],
                    wsb[:, ct, hp * 128:(hp + 1) * 128],
                    xs[:, ct, ih * 512:(ih + 1) * 512],
                    start=(ct == 0), stop=(ct == CT - 1),
                )
            if ih == 1:
                nc.vector.tensor_copy(dst[:], ps[:])
                del proj_state[key]
            return dst

        def emit_v_proj_half(b, jt, eh):
            """Half (512 e-columns) of the V[j,e] projection for j-tile jt."""
            key = ("v", b, jt)
            if eh == 0:
                ps = pss.tile([128, INNER], f32, tag="sm", name=f"ps_v{b}_{jt}")
                proj_state[key] = ps
            ps = proj_state[key]
            xs = x_tiles[b]
            for ct in range(CT):
                nc.tensor.matmul(
                    ps[:, eh * 512:(eh + 1) * 512],
                    xs[:, ct, jt * 128:(jt + 1) * 128],
                    wv_sb[:, ct, eh * 512:(eh + 1) * 512],
                    start=(ct == 0), stop=(ct == CT - 1),
                )
            if eh == 1:
                nc.vector.tensor_copy(v_tiles[b][:, jt], ps[:])
                del proj_state[key]

        def emit_tail_transposes(b, w_rows):
            wT = sp.tile([128, NT, HEADS], bf16, tag="wt", bufs=2, name=f"wT{b}")
            for jt in range(NT):
                tp = pss.tile([128, HEADS], bf16, tag="sm", name=f"tp{b}_{jt}")
                nc.tensor.transpose(tp[:], w_rows[:, jt * 128:(jt + 1) * 128],
                                    ident[:])
                nc.vector.tensor_copy(wT[:, jt], tp[:])
            return wT

        def emit_tail_fin(b, wT):
            fin = pss.tile([HEADS, INNER], f32, tag="sm", name=f"fin{b}")
            for eh in range(2):
                for jt in range(NT):
                    nc.tensor.matmul(
                        fin[:, eh * 512:(eh + 1) * 512],
                        wT[:, jt],
                        v_tiles[b][:, jt, eh * 512:(eh + 1) * 512],
                        start=(jt == 0), stop=(jt == NT - 1),
                    )
            fin_sb = sp.tile([HEADS, INNER], f32, tag="finsb", bufs=2,
                             name=f"finsb{b}")
            nc.vector.tensor_scalar_mul(fin_sb[:], fin[:], 1.0 / HW)
            for h in range(HEADS):
                nc.sync.dma_start(
                    out=y_sb[b * HEADS + h:b * HEADS + h + 1, :],
                    in_=fin_sb[h:h + 1, h * HEAD_DIM:(h + 1) * HEAD_DIM])
            del v_tiles[b]

        # ---- startup ----
        emit_x(0)
        emit_qk_proj_half(0, 0, wq_sb, "qt", 0)
        qt_tiles[(0, 0)] = emit_qk_proj_half(0, 0, wq_sb, "qt", 1)
        emit_qk_proj_half(0, 0, wk_sb, "kt", 0)
        kt_tiles[(0, 0)] = emit_qk_proj_half(0, 0, wk_sb, "kt", 1)

        w_rows_of = {}
        for b in range(B_LOC):
            v_tiles[b] = vp.tile([128, NT, INNER], bf16, tag="v", bufs=2,
                                 name=f"v{b}")
            w_rows = sp.tile([HEADS, HW], bf16, tag="wr", bufs=2, name=f"wr{b}")
            w_rows_of[b] = w_rows
            for hp in range(HP):
                qt = qt_tiles.pop((b, hp))
                kt = kt_tiles.pop((b, hp))
                # next pair to prefetch (same sample, or first pair of next)
                if hp + 1 < HP:
                    nxt = (b, hp + 1)
                elif b + 1 < B_LOC:
                    nxt = (b + 1, 0)
                else:
                    nxt = None
                ex_tiles = {}
                den_t = {}
                for h in range(2):
                    den_t[h] = sp.tile([128, NT], f32, tag="den",
                                       name=f"den{b}_{hp}_{h}")
                for it in range(NT):
                    # --- prefetch / tail injections in half-size chunks,
                    # never at it==0 so the pair's first scores reach ACT
                    # immediately ---
                    if nxt is not None:
                        if it == 1:
                            emit_qk_proj_half(nxt[0], nxt[1], wq_sb, "qt", 0)
                        if it == 2:
                            qt_tiles[nxt] = emit_qk_proj_half(
                                nxt[0], nxt[1], wq_sb, "qt", 1)
                        if it == 3:
                            emit_qk_proj_half(nxt[0], nxt[1], wk_sb, "kt", 0)
                        if it == 4:
                            kt_tiles[nxt] = emit_qk_proj_half(
                                nxt[0], nxt[1], wk_sb, "kt", 1)
                    if it == 2 and hp == 0 and b + 1 < B_LOC:
                        emit_x(b + 1)
                    if hp >= 1:
                        if it == 5:
                            emit_v_proj_half(b, hp - 1, 0)
                        if it == 6:
                            emit_v_proj_half(b, hp - 1, 1)
                    if hp == HP - 1 and it == 7:
                        emit_v_proj_half(b, NT - 1, 0)
                    # previous sample's tail hides inside this sample's pair 0
                    if hp == 0 and b >= 1:
                        if it == 5:
                            wT_prev = emit_tail_transposes(b - 1, w_rows_of[b - 1])
                        if it == 7:
                            emit_tail_fin(b - 1, wT_prev)
                    # --- scores for both heads (distinct PE row groups) ---
                    s0 = psb.tile([128, HW], f32, tag="big", name=f"s0_{b}_{hp}_{it}")
                    s1 = psb.tile([128, HW], f32, tag="big", name=f"s1_{b}_{hp}_{it}")
                    # alternate heads so each MM overlaps its row-group partner
                    for jh in range(2):
                        for h, s in ((0, s0), (1, s1)):
                            nc.tensor.matmul(
                                s[:, jh * 512:(jh + 1) * 512],
                                qt[h * 64:(h + 1) * 64, it * 128:(it + 1) * 128],
                                kt[h * 64:(h + 1) * 64, jh * 512:(jh + 1) * 512],
                                start=True, stop=True,
                            )
                    # --- exp with fused row-sum into den column `it` ---
                    for h, s in ((0, s0), (1, s1)):
                        ex = ep.tile([128, HW], bf16, tag="e",
                                     name=f"e{b}_{hp}_{h}_{it}")
                        nc.scalar.activation(ex[:], s[:], AF.Exp, scale=SCALE,
                                             accum_out=den_t[h][:, it:it + 1])
                        ex_tiles[(h, it)] = ex
                if hp == HP - 1:
                    emit_v_proj_half(b, NT - 1, 1)
                # --- pair-end: r = 1/den, then the dense w block ---
                # (h, jh) half goes to psum row 32*(2h+jh): 4 distinct PE
                # column groups, so all four M=1 matmuls run concurrently
                rb_t = {}
                for h in range(2):
                    r = sp.tile([128, NT], f32, tag="r", name=f"r{b}_{hp}_{h}")
                    # pad rb columns to 4 bytes so each [128,1] weight slice
                    # for the PE stays 4B-aligned
                    rb = sp.tile([128, NT, 2], bf16, tag="rb",
                                 name=f"rb{b}_{hp}_{h}")
                    nc.vector.reciprocal(r[:], den_t[h][:])
                    nc.vector.tensor_copy(rb[:, :, 0], r[:])
                    rb_t[h] = rb
                w_ps = psb.tile([128, HW], f32, tag="big", name=f"w{b}_{hp}")
                for it in range(NT):
                    for h in range(2):
                        for jh in range(2):
                            row = 32 * (2 * h + jh)
                            nc.tensor.matmul(
                                w_ps[row:row + 1, jh * 512:(jh + 1) * 512],
                                rb_t[h][:, it, 0:1],
                                ex_tiles[(h, it)][:, jh * 512:(jh + 1) * 512],
                                start=(it == 0), stop=(it == NT - 1),
                                skip_group_check=True,
                                tile_position=(0, row),
                            )
                stage = sp.tile([128, HW], bf16, tag="wstage", bufs=2,
                                name=f"wstage{b}_{hp}")
                nc.vector.tensor_copy(stage[:], w_ps[:, :])
                for h in range(2):
                    for jh in range(2):
                        row = 32 * (2 * h + jh)
                        nc.sync.dma_start(
                            out=w_rows[2 * hp + h:2 * hp + h + 1,
                                       jh * 512:(jh + 1) * 512],
                            in_=stage[row:row + 1, jh * 512:(jh + 1) * 512])

        # last sample's tail (nothing left to hide it behind)
        wT_last = emit_tail_transposes(B_LOC - 1, w_rows_of[B_LOC - 1])
        emit_tail_fin(B_LOC - 1, wT_last)

        # ---- LayerNorm over last dim (64) for all 64 (b,h) rows ----
        P = B_LOC * HEADS
        stats = sp.tile([P, 6], f32, tag="st", bufs=1, name="stats")
        mv = sp.tile([P, 2], f32, tag="mv", bufs=1, name="mv")
        std = sp.tile([P, 1], f32, tag="sd", bufs=1, name="std")
        nc.vector.bn_stats(stats[:], y_sb[:])
        nc.vector.bn_aggr(mv[:], stats[:])
        nc.scalar.activation(std[:], mv[:, 1:2], AF.Sqrt,
                             bias=eps_sb[:], scale=1.0)
        nc.vector.reciprocal(std[:], std[:])
        nc.vector.tensor_scalar(y_sb[:], y_sb[:], mv[:, 0:1], std[:],
                                op0=Alu.subtract, op1=Alu.mult)
        nc.vector.tensor_mul(y_sb[:], y_sb[:], gam_sb[:])
        nc.vector.tensor_add(y_sb[:], y_sb[:], bet_sb[:])
        nc.sync.dma_start(out=y_d, in_=y_sb[:])

    nc.compile()
    return nc


def _get_nc():
    if "nc" not in _CACHE:
        _CACHE["nc"] = _build_module()
    return _CACHE["nc"]


def _prep_in_maps(x, Wq, Wk, Wv, gamma, beta):
    import ml_dtypes
    bf = ml_dtypes.bfloat16
    x = np.asarray(x, np.float32)
    wqT = np.ascontiguousarray(np.asarray(Wq, np.float32).T.astype(bf))
    wkT = np.ascontiguousarray(np.asarray(Wk, np.float32).T.astype(bf))
    wvT = np.ascontiguousarray(np.asarray(Wv, np.float32).T.astype(bf))
    gam2 = np.ascontiguousarray(
        np.broadcast_to(np.asarray(gamma, np.float32), (B_LOC * HEADS, HEAD_DIM)))
    bet2 = np.ascontiguousarray(
        np.broadcast_to(np.asarray(beta, np.float32), (B_LOC * HEADS, HEAD_DIM)))
    in_maps = []
    for c in range(N_CORES):
        xb = np.ascontiguousarray(
            x[c * B_LOC:(c + 1) * B_LOC].reshape(B_LOC, C, HW).astype(bf))
        in_maps.append(dict(x=xb, wqT=wqT, wkT=wkT, wvT=wvT,
                            gamma2d=gam2, beta2d=bet2))
    return in_maps


def _run(inputs, trace=False):
    from concourse.bass_utils import run_bass_kernel_spmd
    nc = _get_nc()
    in_maps = _prep_in_maps(**inputs)
    res = run_bass_kernel_spmd(nc, in_maps, core_ids=list(range(N_CORES)),
                               trace=trace)
    out = np.concatenate(
        [np.asarray(res.results[c]["y"], np.float32).reshape(B_LOC, HEADS, HEAD_DIM)
         for c in range(N_CORES)],
        axis=0)
    return out, res


def kernel(x, Wq, Wk, Wv, gamma, beta):
    out, _ = _run(dict(x=x, Wq=Wq, Wk=Wk, Wv=Wv, gamma=gamma, beta=beta))
    return out


# revision 24
# speedup vs baseline: 1.3520x; 1.0124x over previous
"""Trainium2 Bass kernel for nn_AttentionMLP: per-sample 16-head attention over
N=1024 tokens with mean-pooling + LayerNorm.  Data-parallel over batch across
8 NeuronCores (4 samples/core).

Key algebraic restructuring: the reference computes
    out = mean_i( softmax(q_i K^T * s) @ V );  y = LN(out)
By linearity of the mean, with e[i,j] = exp(s * S[i,j]) and den[i] = sum_j e[i,j]:
    out = (1/N) * (sum_i e[i,:] / den[i]) @ V = (1/N) * w @ V
so the [N,N]@[N,64] attention-value matmul collapses to a rank-1 reduction
(w = r^T @ e, an M=1 matmul on the PE) plus one [1,N]@[N,64] product.
The exp of all N^2 scores (the unavoidable cost) runs on the scalar engine
with the fused per-row accumulate (accum_out) producing den for free.

Precision: matmuls run in bf16 (fp32 runs the PE at ~5x lower effective
throughput: 2 HW passes x half stream rate); PSUM accumulation, den,
reciprocal and the LayerNorm are fp32.  Errors injected on exp/r average
out over the 1024-token reduction before reaching the output.

Layouts (per core):
  x_sb  [128c, 5ct, 1024i]   (c = ct*128 + p), straight from DRAM
  qT/kT per head-pair [128e', 1024i] via matmul(lhsT=W*T[c,e], rhs=x[c,i])
  scores S[i,j] psum [128, 1024] per (head, i-tile); 2 heads packed in
  distinct PE row groups (K=64).  w accumulated in psum [1,1024] rows at
  col-group 0/32 (concurrent).  V[j,e] per sample, fin = wT^T @ V.
"""

import numpy as np

HEADS = 16
HEAD_DIM = 64
B, C, HW = 32, 640, 1024
N_CORES = 8
B_LOC = B // N_CORES      # 4 samples per core
CT = C // 128             # 5 contraction tiles
NT = HW // 128            # 8 token tiles
HP = HEADS // 2           # 8 head pairs
INNER = HEADS * HEAD_DIM  # 1024
LN_EPS = 1e-5
SCALE = HEAD_DIM ** -0.5

_CACHE = {}


def _build_module():
    from contextlib import ExitStack
    import concourse.bass as bass
    import concourse.bacc as bacc
    import concourse.mybir as mybir
    import concourse.tile as tile
    from concourse import masks

    f32 = mybir.dt.float32
    bf16 = mybir.dt.bfloat16
    AF = mybir.ActivationFunctionType
    Alu = mybir.AluOpType

    nc = bacc.Bacc("TRN2", debug=False, enable_asserts=False)

    x_d = nc.dram_tensor("x", [B_LOC, C, HW], bf16, kind="ExternalInput").ap()
    wq_d = nc.dram_tensor("wqT", [C, INNER], bf16, kind="ExternalInput").ap()
    wk_d = nc.dram_tensor("wkT", [C, INNER], bf16, kind="ExternalInput").ap()
    wv_d = nc.dram_tensor("wvT", [C, INNER], bf16, kind="ExternalInput").ap()
    gam_d = nc.dram_tensor("gamma2d", [B_LOC * HEADS, HEAD_DIM], f32,
                           kind="ExternalInput").ap()
    bet_d = nc.dram_tensor("beta2d", [B_LOC * HEADS, HEAD_DIM], f32,
                           kind="ExternalInput").ap()
    y_d = nc.dram_tensor("y", [B_LOC * HEADS, HEAD_DIM], f32,
                         kind="ExternalOutput").ap()
    # DRAM bounce buffer for the block-diagonal extract of fin (a diagonal
    # is not an affine SBUF access pattern, but is affine in DRAM)
    scr_d = nc.dram_tensor("scr", [B_LOC, HEADS * INNER], f32).ap()

    with tile.TileContext(nc) as tc, ExitStack() as ctx:
        wts = ctx.enter_context(tc.tile_pool(name="wts", bufs=1))
        xp = ctx.enter_context(tc.tile_pool(name="xp", bufs=2))
        vp = ctx.enter_context(tc.tile_pool(name="vp", bufs=1))
        qkp = ctx.enter_context(tc.tile_pool(name="qkp", bufs=2))
        ep = ctx.enter_context(tc.tile_pool(name="ep", bufs=20))
        sp = ctx.enter_context(tc.tile_pool(name="sp", bufs=4))
        # scores triple-buffer (+ transient w block): 3 x [128,1024]f32 = 6 banks
        psb = ctx.enter_context(tc.tile_pool(name="psb", bufs=3, space="PSUM"))
        # projections / transposes / final: 1 x 2 banks
        pss = ctx.enter_context(tc.tile_pool(name="pss", bufs=1, space="PSUM"))

        # ---- constants / weights ----
        wq_sb = wts.tile([128, CT, INNER], bf16, tag="wq", name="wq_sb")
        wk_sb = wts.tile([128, CT, INNER], bf16, tag="wk", name="wk_sb")
        wv_sb = wts.tile([128, CT, INNER], bf16, tag="wv", name="wv_sb")
        for wsb, wd in ((wq_sb, wq_d), (wk_sb, wk_d)):
            wr = wd.rearrange("(ct p) e -> ct p e", p=128)
            for ct in range(CT):
                nc.sync.dma_start(out=wsb[:, ct], in_=wr[ct])

        ident = wts.tile([16, 16], bf16, tag="ident", name="ident")
        masks.make_identity(nc, ident[:])
        # (engine APs must start at a partition multiple of 32; per-head row
        # scatter/gather below therefore goes through SBUF->SBUF DMA)
        gam_sb = wts.tile([B_LOC * HEADS, HEAD_DIM], f32, tag="gam", name="gam_sb")
        bet_sb = wts.tile([B_LOC * HEADS, HEAD_DIM], f32, tag="bet", name="bet_sb")
        nc.sync.dma_start(out=gam_sb[:], in_=gam_d)
        nc.sync.dma_start(out=bet_sb[:], in_=bet_d)
        eps_sb = wts.tile([B_LOC * HEADS, 1], f32, tag="eps", name="eps_sb")
        nc.vector.memset(eps_sb[:], LN_EPS)

        y_sb = wts.tile([B_LOC * HEADS, HEAD_DIM], f32, tag="y", name="y_sb")

        x_tiles = {}
        qt_tiles = {}
        kt_tiles = {}
        v_tiles = {}

        def emit_x(b):
            xs = xp.tile([128, CT, HW], bf16, tag="x", name=f"x{b}")
            xr = x_d[b].rearrange("(ct p) i -> ct p i", p=128)
            for ct in range(CT):
                nc.sync.dma_start(out=xs[:, ct], in_=xr[ct])
            x_tiles[b] = xs

        proj_state = {}

        def emit_qk_proj_half(b, hp, wsb, which, ih):
            """Half (512 i-columns) of the qT/kT projection for pair hp.
            Emitted in two chunks so the PE detour never starves ACT."""
            key = (which, b, hp)
            if ih == 0:
                dst = qkp.tile([128, HW], bf16, tag=which, name=f"{which}{b}_{hp}")
                ps = pss.tile([128, HW], f32, tag="sm", name=f"ps_{which}{b}_{hp}")
                proj_state[key] = (dst, ps)
            dst, ps = proj_state[key]
            xs = x_tiles[b]
            for ct in range(CT):
                nc.tensor.matmul(
                    ps[:, ih * 512:(ih + 1) * 512],
                    wsb[:, ct, hp * 128:(hp + 1) * 128],
                    xs[:, ct, ih * 512:(ih + 1) * 512],
                    start=(ct == 0), stop=(ct == CT - 1),
                )
            if ih == 1:
                nc.vector.tensor_copy(dst[:], ps[:])
                del proj_state[key]
            return dst

        def emit_v_proj_half(b, jt, eh):
            """Half (512 e-columns) of the V[j,e] projection for j-tile jt."""
            key = ("v", b, jt)
            if eh == 0:
                ps = pss.tile([128, INNER], f32, tag="sm", name=f"ps_v{b}_{jt}")
                proj_state[key] = ps
            ps = proj_state[key]
            xs = x_tiles[b]
            for ct in range(CT):
                nc.tensor.matmul(
                    ps[:, eh * 512:(eh + 1) * 512],
                    xs[:, ct, jt * 128:(jt + 1) * 128],
                    wv_sb[:, ct, eh * 512:(eh + 1) * 512],
                    start=(ct == 0), stop=(ct == CT - 1),
                )
            if eh == 1:
                nc.vector.tensor_copy(v_tiles[b][:, jt], ps[:])
                del proj_state[key]

        tail_state = {}

        def emit_tail_transposes(b, w_rows, half):
            if half == 0:
                tail_state[("wt", b)] = sp.tile([128, NT, HEADS], bf16,
                                                tag="wt", bufs=2, name=f"wT{b}")
            wT = tail_state[("wt", b)]
            for jt in range(half * 4, half * 4 + 4):
                tp = pss.tile([128, HEADS], bf16, tag="sm", name=f"tp{b}_{jt}")
                nc.tensor.transpose(tp[:], w_rows[:, jt * 128:(jt + 1) * 128],
                                    ident[:])
                nc.vector.tensor_copy(wT[:, jt], tp[:])
            return wT

        def emit_tail_fin(b, wT, eh):
            if eh == 0:
                tail_state[("fin", b)] = pss.tile([HEADS, INNER], f32,
                                                  tag="sm", name=f"fin{b}")
            fin = tail_state[("fin", b)]
            for jt in range(NT):
                nc.tensor.matmul(
                    fin[:, eh * 512:(eh + 1) * 512],
                    wT[:, jt],
                    v_tiles[b][:, jt, eh * 512:(eh + 1) * 512],
                    start=(jt == 0), stop=(jt == NT - 1),
                )
            if eh == 1:
                fin_sb = sp.tile([HEADS, INNER], f32, tag="finsb", bufs=2,
                                 name=f"finsb{b}")
                nc.vector.tensor_scalar_mul(fin_sb[:], fin[:], 1.0 / HW)
                # block-diagonal extract via DRAM bounce (2 DMAs, not 16)
                nc.sync.dma_start(out=scr_d[b].rearrange("(h e) -> h e", h=HEADS),
                                  in_=fin_sb[:])
                diag = bass.AP(tensor=scr_d.tensor, offset=b * HEADS * INNER,
                               ap=[[INNER + HEAD_DIM, HEADS], [1, HEAD_DIM]])
                nc.sync.dma_start(
                    out=y_sb[b * HEADS:(b + 1) * HEADS, :], in_=diag)
                del v_tiles[b]
                del tail_state[("wt", b)]
                del tail_state[("fin", b)]

        # ---- startup (only wq/wk/x DMAs precede the first projections;
        # wv and LN constants are emitted after so they don't delay them) ----
        emit_x(0)
        emit_qk_proj_half(0, 0, wq_sb, "qt", 0)
        qt_tiles[(0, 0)] = emit_qk_proj_half(0, 0, wq_sb, "qt", 1)
        emit_qk_proj_half(0, 0, wk_sb, "kt", 0)
        kt_tiles[(0, 0)] = emit_qk_proj_half(0, 0, wk_sb, "kt", 1)
        wvr = wv_d.rearrange("(ct p) e -> ct p e", p=128)
        for ct in range(CT):
            nc.sync.dma_start(out=wv_sb[:, ct], in_=wvr[ct])

        w_rows_of = {}
        for b in range(B_LOC):
            v_tiles[b] = vp.tile([128, NT, INNER], bf16, tag="v", bufs=2,
                                 name=f"v{b}")
            w_rows = sp.tile([HEADS, HW], bf16, tag="wr", bufs=2, name=f"wr{b}")
            w_rows_of[b] = w_rows
            for hp in range(HP):
                qt = qt_tiles.pop((b, hp))
                kt = kt_tiles.pop((b, hp))
                # next pair to prefetch (same sample, or first pair of next)
                if hp + 1 < HP:
                    nxt = (b, hp + 1)
                elif b + 1 < B_LOC:
                    nxt = (b + 1, 0)
                else:
                    nxt = None
                ex_tiles = {}
                den_t = {}
                for h in range(2):
                    den_t[h] = sp.tile([128, NT], f32, tag="den",
                                       name=f"den{b}_{hp}_{h}")
                for it in range(NT):
                    # --- prefetch / tail injections in half-size chunks,
                    # never at it==0 so the pair's first scores reach ACT
                    # immediately ---
                    if nxt is not None:
                        if it == 1:
                            emit_qk_proj_half(nxt[0], nxt[1], wq_sb, "qt", 0)
                        if it == 2:
                            qt_tiles[nxt] = emit_qk_proj_half(
                                nxt[0], nxt[1], wq_sb, "qt", 1)
                        if it == 3:
                            emit_qk_proj_half(nxt[0], nxt[1], wk_sb, "kt", 0)
                        if it == 4:
                            kt_tiles[nxt] = emit_qk_proj_half(
                                nxt[0], nxt[1], wk_sb, "kt", 1)
                    if it == 2 and hp == 0 and b + 1 < B_LOC:
                        emit_x(b + 1)
                    if hp >= 1:
                        if it == 5:
                            emit_v_proj_half(b, hp - 1, 0)
                        if it == 6:
                            emit_v_proj_half(b, hp - 1, 1)
                    if hp == HP - 1 and it == 7:
                        emit_v_proj_half(b, NT - 1, 0)
                    # previous sample's tail hides inside this sample's pair 0
                    if hp == 0 and b >= 1:
                        if it == 4:
                            wT_prev = emit_tail_transposes(
                                b - 1, w_rows_of[b - 1], 0)
                        if it == 5:
                            emit_tail_transposes(b - 1, w_rows_of[b - 1], 1)
                        if it == 6:
                            emit_tail_fin(b - 1, wT_prev, 0)
                        if it == 7:
                            emit_tail_fin(b - 1, wT_prev, 1)
                    # --- scores for both heads (distinct PE row groups) ---
                    s0 = psb.tile([128, HW], f32, tag="big", name=f"s0_{b}_{hp}_{it}")
                    s1 = psb.tile([128, HW], f32, tag="big", name=f"s1_{b}_{hp}_{it}")
                    # alternate heads so each MM overlaps its row-group partner
                    for jh in range(2):
                        for h, s in ((0, s0), (1, s1)):
                            nc.tensor.matmul(
                                s[:, jh * 512:(jh + 1) * 512],
                                qt[h * 64:(h + 1) * 64, it * 128:(it + 1) * 128],
                                kt[h * 64:(h + 1) * 64, jh * 512:(jh + 1) * 512],
                                start=True, stop=True,
                            )
                    # --- exp with fused row-sum into den column `it` ---
                    for h, s in ((0, s0), (1, s1)):
                        ex = ep.tile([128, HW], bf16, tag="e",
                                     name=f"e{b}_{hp}_{h}_{it}")
                        nc.scalar.activation(ex[:], s[:], AF.Exp, scale=SCALE,
                                             accum_out=den_t[h][:, it:it + 1])
                        ex_tiles[(h, it)] = ex
                if hp == HP - 1:
                    emit_v_proj_half(b, NT - 1, 1)
                # --- pair-end: r = 1/den, then the dense w block ---
                # (h, jh) half goes to psum row 32*(2h+jh): 4 distinct PE
                # column groups, so all four M=1 matmuls run concurrently
                rb_t = {}
                for h in range(2):
                    r = sp.tile([128, NT], f32, tag="r", name=f"r{b}_{hp}_{h}")
                    # pad rb columns to 4 bytes so each [128,1] weight slice
                    # for the PE stays 4B-aligned
                    rb = sp.tile([128, NT, 2], bf16, tag="rb",
                                 name=f"rb{b}_{hp}_{h}")
                    nc.vector.reciprocal(r[:], den_t[h][:])
                    nc.vector.tensor_copy(rb[:, :, 0], r[:])
                    rb_t[h] = rb
                w_ps = psb.tile([128, HW], f32, tag="big", name=f"w{b}_{hp}")
                for it in range(NT):
                    for h in range(2):
                        for jh in range(2):
                            row = 32 * (2 * h + jh)
                            nc.tensor.matmul(
                                w_ps[row:row + 1, jh * 512:(jh + 1) * 512],
                                rb_t[h][:, it, 0:1],
                                ex_tiles[(h, it)][:, jh * 512:(jh + 1) * 512],
                                start=(it == 0), stop=(it == NT - 1),
                                skip_group_check=True,
                                tile_position=(0, row),
                            )
                stage = sp.tile([128, HW], bf16, tag="wstage", bufs=2,
                                name=f"wstage{b}_{hp}")
                nc.vector.tensor_copy(stage[:], w_ps[:, :])
                for h in range(2):
                    for jh in range(2):
                        row = 32 * (2 * h + jh)
                        nc.sync.dma_start(
                            out=w_rows[2 * hp + h:2 * hp + h + 1,
                                       jh * 512:(jh + 1) * 512],
                            in_=stage[row:row + 1, jh * 512:(jh + 1) * 512])

        # last sample's tail (nothing left to hide it behind)
        wT_last = emit_tail_transposes(B_LOC - 1, w_rows_of[B_LOC - 1], 0)
        emit_tail_transposes(B_LOC - 1, w_rows_of[B_LOC - 1], 1)
        emit_tail_fin(B_LOC - 1, wT_last, 0)
        emit_tail_fin(B_LOC - 1, wT_last, 1)

        # ---- LayerNorm over last dim (64) for all 64 (b,h) rows ----
        P = B_LOC * HEADS
        stats = sp.tile([P, 6], f32, tag="st", bufs=1, name="stats")
        mv = sp.tile([P, 2], f32, tag="mv", bufs=1, name="mv")
        std = sp.tile([P, 1], f32, tag="sd", bufs=1, name="std")
        nc.vector.bn_stats(stats[:], y_sb[:])
        nc.vector.bn_aggr(mv[:], stats[:])
        nc.scalar.activation(std[:], mv[:, 1:2], AF.Sqrt,
                             bias=eps_sb[:], scale=1.0)
        nc.vector.reciprocal(std[:], std[:])
        nc.vector.tensor_scalar(y_sb[:], y_sb[:], mv[:, 0:1], std[:],
                                op0=Alu.subtract, op1=Alu.mult)
        nc.vector.tensor_mul(y_sb[:], y_sb[:], gam_sb[:])
        nc.vector.tensor_add(y_sb[:], y_sb[:], bet_sb[:])
        nc.sync.dma_start(out=y_d, in_=y_sb[:])

    nc.compile()
    return nc


def _get_nc():
    if "nc" not in _CACHE:
        _CACHE["nc"] = _build_module()
    return _CACHE["nc"]


def _prep_in_maps(x, Wq, Wk, Wv, gamma, beta):
    import ml_dtypes
    bf = ml_dtypes.bfloat16
    x = np.asarray(x, np.float32)
    wqT = np.ascontiguousarray(np.asarray(Wq, np.float32).T.astype(bf))
    wkT = np.ascontiguousarray(np.asarray(Wk, np.float32).T.astype(bf))
    wvT = np.ascontiguousarray(np.asarray(Wv, np.float32).T.astype(bf))
    gam2 = np.ascontiguousarray(
        np.broadcast_to(np.asarray(gamma, np.float32), (B_LOC * HEADS, HEAD_DIM)))
    bet2 = np.ascontiguousarray(
        np.broadcast_to(np.asarray(beta, np.float32), (B_LOC * HEADS, HEAD_DIM)))
    in_maps = []
    for c in range(N_CORES):
        xb = np.ascontiguousarray(
            x[c * B_LOC:(c + 1) * B_LOC].reshape(B_LOC, C, HW).astype(bf))
        in_maps.append(dict(x=xb, wqT=wqT, wkT=wkT, wvT=wvT,
                            gamma2d=gam2, beta2d=bet2))
    return in_maps


def _run(inputs, trace=False):
    from concourse.bass_utils import run_bass_kernel_spmd
    nc = _get_nc()
    in_maps = _prep_in_maps(**inputs)
    res = run_bass_kernel_spmd(nc, in_maps, core_ids=list(range(N_CORES)),
                               trace=trace)
    out = np.concatenate(
        [np.asarray(res.results[c]["y"], np.float32).reshape(B_LOC, HEADS, HEAD_DIM)
         for c in range(N_CORES)],
        axis=0)
    return out, res


def kernel(x, Wq, Wk, Wv, gamma, beta):
    out, _ = _run(dict(x=x, Wq=Wq, Wk=Wk, Wv=Wv, gamma=gamma, beta=beta))
    return out
